# revision 33
# baseline (speedup 1.0000x reference)
"""Trainium2 Bass kernel for BinaryHead: logits = (l2norm(fea) @ W.T + b) * 16.

Data-parallel over batch across 8 NeuronCores (2048 rows each), with the
shard staged TRANSPOSED ([emb, batch]) so the contraction dim lands on SBUF
partitions.

fp8 design: the shard is staged as fp8e4m3 (4.2MB/core, half the bf16 HBM
traffic).  Plain fp8 rounding costs ~1.6e-2 relative error; instead the host
picks each element's rounding direction by greedy error diffusion against the
effective W so the per-row residual <delta, W_c> stays near zero
(l2rel ~2.6e-3).  The per-row scale that a quantizer would want cancels in
z/sqrt(sumsq), so no scales are needed on device.

W rides as an fp8 hi/lo column pair: Whi = fp8(32*W), Wlo = fp8(16*resid),
combined on device as z = (zhi + zlo/16)/32 -- the 1/32 folds into the Rsqrt
scale.  Per core the device kernel runs, per e-panel pair [256e x 2048b]:

  z.T[0:8, b]  += [Whi|Wlo].T @ panelpair      (fp8 DoubleRow, psum strip 0)
  sumsq[b]     += ones.T @ fp8(panel**2)       (fp8 DoubleRow, psum strip 1)

Squares are split per-panel across ACT/DVE/GPSIMD.  The ss matmuls write at
psum partition 32 (a different PE column strip than z) so the two DR streams
can overlap in the array.  Epilogue per 512-col chunk: rnorm = Rsqrt(4*ss)
(one ACT op; table reciprocal_sqrt_and_small covers Square+Rsqrt so no table
switch), rnb = ones1x4 @ rnorm (k=1 matmul broadcast), z = (zlo/16 + zhi)
via scalar_tensor_tensor, out = z*rnb + S*b, DMA out.

Head: PE warmup matmuls run off memset tiles (no DMA dependency) so HAM
un-throttles early, and the first feaT chunks are the first transfers on the
sync ring (consts ride the scalar ring) so data lands at ~2us not ~8us.
"""

import os
from contextlib import ExitStack

import numpy as np

NUM_CLASS = 4
EMB = 2048
BATCH = 16384
N_CORES = 8
ROWS = BATCH // N_CORES  # 2048 rows per core
S = 16.0
WS = 32.0  # W pre-scale before fp8 split

N_ETILES = EMB // 128  # 16 e-panels per core
N_PAIRS = N_ETILES // 2  # 8 DoubleRow pairs
N_BCHUNK = ROWS // 512  # 4 psum-width chunks of the batch

# per-panel square engine: A=ACT, V=DVE, G=GPSIMD (16 chars)
SQ_ENGINES = os.environ.get("KERNEL_SQ", "ACACACACACACACAV")
SS_STRIP1 = os.environ.get("KERNEL_SS_STRIP1", "1") == "1"
N_WARMUP = int(os.environ.get("KERNEL_WARMUP", "10"))

_CACHE = {}


def _build_nc():
    import concourse.bacc as bacc
    import concourse.mybir as mybir
    import concourse.tile as tile
    from concourse.hw_specs import get_activation_tables

    f32 = mybir.dt.float32
    f32r = mybir.dt.float32r
    fp8 = mybir.dt.float8e4
    u8 = mybir.dt.uint8  # fp8 bytes cross the host interface as uint8
    DR = mybir.MatmulPerfMode.DoubleRow

    nc = bacc.Bacc(
        "TRN2",
        target_bir_lowering=False,
        debug=False,
        enable_asserts=False,
        num_devices=N_CORES,
    )

    feaT = nc.dram_tensor("feaT", [EMB, ROWS], u8, kind="ExternalInput").ap()
    # wt[p, pair, ktile, 0:4]=Whi, [4:8]=Wlo, [8:16]=0 (pad keeps the DR
    # ktile step %16==0).  One DR matmul -> z hi/lo at psum partitions 0-7.
    wt = nc.dram_tensor("wt", [128, N_PAIRS, 2, 16], u8, kind="ExternalInput").ap()
    onesv = nc.dram_tensor("onesv", [128, 2, 16], u8, kind="ExternalInput").ap()
    sones = nc.dram_tensor("sones", [1, 8], f32r, kind="ExternalInput").ap()
    # selector: rows 0-3 pick z_hi, rows 4-7 z_lo/16, row 32 hits the
    # ones-row of zr -> adds S*b.  Cross-partition sums live on the PE; no
    # engine ever reads at an unaligned partition base.
    selw = nc.dram_tensor("selw", [33, NUM_CLASS], f32r, kind="ExternalInput").ap()
    outT = nc.dram_tensor("outT", [NUM_CLASS, ROWS], f32, kind="ExternalOutput").ap()

    with tile.TileContext(nc) as tc, ExitStack() as ctx:
        pconst = ctx.enter_context(tc.tile_pool(name="pconst", bufs=1))
        pdata = ctx.enter_context(tc.tile_pool(name="pdata", bufs=5))
        psq = ctx.enter_context(tc.tile_pool(name="psq", bufs=5))
        pxb = ctx.enter_context(tc.tile_pool(name="pxb", bufs=5))
        pep = ctx.enter_context(tc.tile_pool(name="pep", bufs=1))
        pz = ctx.enter_context(tc.tile_pool(name="pz", bufs=1, space="PSUM"))
        ps = ctx.enter_context(tc.tile_pool(name="ps", bufs=1, space="PSUM"))

        # preload the ACT table covering Square+Ln+Exp before any activation
        nlx_id = list(get_activation_tables(nc.m.arch)).index(
            "natural_log_exp_and_others"
        )
        nc.scalar.add_instruction(
            mybir.InstLoadActFuncSet(name=f"I-{nc.next_id()}", act_func_set_id=nlx_id)
        )

        # warmup operands: memset tiles, no DMA dependency -> PE busy from t~0
        wu_w = pconst.tile([128, NUM_CLASS], fp8)
        nc.vector.memset(wu_w, 0.0)
        wu_x = pconst.tile([128, 512], fp8)
        nc.vector.memset(wu_x, 0.0)
        zero1_s = pconst.tile([1, 1], f32)
        nc.vector.memset(zero1_s, 0.0)
        zero128_s = pconst.tile([128, 1], f32)
        nc.vector.memset(zero128_s, 0.0)
        # rsqrt via exp(-0.5*ln(ss) + ln(S/WS)): folds the output scale in
        lnS_s = pconst.tile([1, 1], f32)
        nc.vector.memset(lnS_s, float(np.log(S / WS)))

        # consts ride the front of the SCALAR ring (feaT owns the sync ring)
        wt_s = pconst.tile([128, N_PAIRS, 2, 16], u8)
        nc.scalar.dma_start(out=wt_s, in_=wt)
        ones_s = pconst.tile([128, 2, 16], u8)
        nc.scalar.dma_start(out=ones_s, in_=onesv)
        sones_s = pconst.tile([1, 8], f32r)
        nc.gpsimd.dma_start(out=sones_s, in_=sones)
        selw_s = pconst.tile([33, NUM_CLASS], f32r)
        nc.gpsimd.dma_start(out=selw_s, in_=selw)

        # accumulators: z hi/lo [8, 2048] (banks 0-3); sumsq as four
        # single-bank tiles at partition 0 (DoubleRow matmuls must target
        # column strip 0, so no strip spreading).  rnb/zc reuse freed ss
        # banks during the epilogue.
        acc = pz.tile([8, ROWS], f32, tag="acc")
        zt = acc[0:8, :]
        ss_of = [
            ps.tile([1, 512], f32, tag="ssrnb", bufs=4, name=f"ss{j}")
            for j in range(N_BCHUNK)
        ]
        rnb = [
            ps.tile([8, 512], f32, tag="ssrnb", bufs=4, name=f"rnb{j}")
            for j in range(N_BCHUNK)
        ]
        zc = [
            ps.tile([NUM_CLASS, 512], f32, tag="ssrnb", bufs=4, name=f"zc{j}")
            for j in range(N_BCHUNK)
        ]
        lnss_s = pep.tile([1, ROWS], f32)
        rnorm_s = pep.tile([1, ROWS], f32r)
        z8_s = pep.tile([8, ROWS], f32)
        zr_s = pep.tile([33, ROWS], f32r)
        # rows 8-31 stay 0 (zero selector weight), row 32 = 1 feeds the bias
        nc.vector.memset(zr_s[0:32, :].bitcast(f32), 0.0)
        nc.vector.memset(zr_s[32:33, :].bitcast(f32), 1.0)
        out_s = pep.tile([NUM_CLASS, ROWS], f32)

        def square(a, bsl, xt, x2, xb, ai):
            # ai = absolute panel index for the engine map.  fp8-input
            # elementwise ops are slow on DVE (~2.8ns/elem) but fast on ACT;
            # mode C casts fp8->bf16 on GPSIMD then squares on DVE, which
            # runs bf16 at the 2x packed rate.
            eng = SQ_ENGINES[ai]
            xin = xt[:, a, bsl].bitcast(fp8)
            if eng == "A":
                nc.scalar.activation(
                    out=x2[:, a, bsl],
                    in_=xin,
                    func=mybir.ActivationFunctionType.Square,
                    bias=zero128_s,
                    scale=1.0,
                )
            elif eng == "V":
                nc.vector.tensor_mul(x2[:, a, bsl], xin, xin)
            else:
                nc.gpsimd.tensor_copy(xb[:, a, bsl], xin)
                nc.vector.tensor_mul(x2[:, a, bsl], xb[:, a, bsl], xb[:, a, bsl])

        def z_mm(pi, j, xt, a2, start, stop):
            # DoubleRow: contracts panel pair (a2, a2+1) of this tile
            bsl = slice(j * 512, (j + 1) * 512)
            nc.tensor.matmul(
                zt[:, bsl],
                wt_s[:, pi, :, 0:8].bitcast(fp8),
                xt[:, a2 : a2 + 2, bsl].bitcast(fp8),
                perf_mode=DR,
                start=start,
                stop=stop,
            )

        def ss_mm(j, x2, start, stop, p2=0):
            bsl = slice(j * 512, (j + 1) * 512)
            nc.tensor.matmul(
                ss_of[j],
                ones_s[:, :, 0:1].bitcast(fp8),
                x2[:, p2 : p2 + 2, bsl],
                perf_mode=DR,
                start=start,
                stop=stop,
            )

        def epilogue_chunk(j):
            bsl = slice(j * 512, (j + 1) * 512)
            # rnorm = (S/WS)/sqrt(ss) = exp(-0.5*ln(ss) + ln(S/WS))
            nc.scalar.activation(
                out=lnss_s[:, bsl],
                in_=ss_of[j],
                func=mybir.ActivationFunctionType.Ln,
                bias=zero1_s,
                scale=1.0,
            )
            nc.scalar.activation(
                out=rnorm_s[:, bsl],
                in_=lnss_s[:, bsl],
                func=mybir.ActivationFunctionType.Exp,
                bias=lnS_s,
                scale=-0.5,
            )
            # broadcast rnorm to 8 partitions: rows 0-3 get rnorm (for z hi),
            # rows 4-7 get rnorm/16 (for z lo) -- the 1/16 rides the sones
            nc.tensor.matmul(rnb[j], sones_s, rnorm_s[:, bsl], start=True, stop=True)
            # psum -> SBUF copy first: DVE may read only one PSUM input
            nc.scalar.activation(
                out=z8_s[:, bsl],
                in_=zt[:, bsl],
                func=mybir.ActivationFunctionType.Copy,
                bias=0.0,
                scale=1.0,
            )
            nc.vector.tensor_mul(zr_s[0:8, bsl], z8_s[:, bsl], rnb[j])
            # selector matmul sums hi+lo across partitions and adds the bias
            # through the ones row
            nc.tensor.matmul(zc[j], selw_s, zr_s[:, bsl], start=True, stop=True)
            nc.vector.tensor_copy(out_s[:, bsl], zc[j])
            nc.sync.dma_start(out=outT[:, bsl], in_=out_s[:, bsl])

        # PE warmup: N=512 matmuls off memset tiles; first real z MM's
        # start=True reset makes the garbage harmless.  Sized to keep the PE
        # busy until the first data lands (~2us) and trip HAM's busy window.
        for _ in range(N_WARMUP):
            nc.tensor.matmul(
                zt[0:NUM_CLASS, 0:512], wu_w, wu_x, start=True, stop=True
            )

        # groups of panel PAIRS: first/last pair column-chunked, middle quads
        groups = [(0,)] + [(1, 2), (3, 4), (5, 6)] + [(N_PAIRS - 1,)]
        for gi, g in enumerate(groups):
            first = gi == 0
            last = gi == len(groups) - 1
            npan = 2 * len(g)
            xt = pdata.tile([128, npan, ROWS], u8, tag="xt")
            x2 = psq.tile([128, npan, ROWS], fp8, tag="x2")
            xb = pxb.tile([128, npan, ROWS], mybir.dt.bfloat16, tag="xb")
            src = feaT[g[0] * 256 : g[0] * 256 + npan * 128, :].rearrange(
                "(a p) b -> p a b", p=128
            )
            # alternate HWDGE rings; feaT chunks lead the sync ring
            dma_eng = nc.sync if gi % 2 == 0 else nc.scalar
            if first or last:
                for j in range(N_BCHUNK):
                    bsl = slice(j * 512, (j + 1) * 512)
                    dma_eng.dma_start(out=xt[:, :, bsl], in_=src[:, :, bsl])
            else:
                dma_eng.dma_start(out=xt, in_=src)

            if first or last:
                pi = g[0]
                for j in range(N_BCHUNK):
                    bsl = slice(j * 512, (j + 1) * 512)
                    square(0, bsl, xt, x2, xb, 2 * pi)
                    square(1, bsl, xt, x2, xb, 2 * pi + 1)
                    z_mm(pi, j, xt, 0, start=first, stop=last)
                if not last:
                    for j in range(N_BCHUNK):
                        ss_mm(j, x2, start=first, stop=last)
                else:
                    for j in range(N_BCHUNK):
                        ss_mm(j, x2, start=first, stop=last)
                        epilogue_chunk(j)
            else:
                full = slice(None)
                for a in range(npan):
                    square(a, full, xt, x2, xb, 2 * g[0] + a)
                # all z first (need only xt), then ss (needs squares) so
                # the in-order PE queue never stalls waiting for squares
                for ai, pi in enumerate(g):
                    for j in range(N_BCHUNK):
                        z_mm(pi, j, xt, 2 * ai, start=False, stop=False)
                for ai, pi in enumerate(g):
                    for j in range(N_BCHUNK):
                        ss_mm(j, x2, start=False, stop=False, p2=2 * ai)

    nc.compile()
    return nc


def _get_nc():
    if "nc" not in _CACHE:
        _CACHE["nc"] = _build_nc()
    return _CACHE["nc"]


def _fp8_neighbors(x):
    """Bracketing fp8e4m3 values (lo <= x <= hi) for each float32 element."""
    import ml_dtypes

    FP8 = ml_dtypes.float8_e4m3fn
    f0 = x.astype(FP8)
    bits = f0.view(np.uint8)
    f0f = f0.astype(np.float32)
    up = np.where(f0f <= x, np.where(x >= 0, bits + 1, bits - 1), bits)
    dn = np.where(f0f <= x, bits, np.where(x >= 0, bits - 1, bits + 1))
    lo = dn.astype(np.uint8).view(FP8).astype(np.float32)
    hi = up.astype(np.uint8).view(FP8).astype(np.float32)
    exact = f0f == x
    lo = np.where(exact, f0f, lo)
    hi = np.where(exact, f0f, hi)
    return np.minimum(lo, hi), np.maximum(lo, hi)


def _diffuse_fp8(fea, Weff):
    """Round fea to fp8, steering each rounding so the running per-row
    residual <delta, Weff_c> stays near zero (greedy error diffusion)."""
    lo, hi = _fp8_neighbors(fea)
    v = np.zeros((fea.shape[0], NUM_CLASS), dtype=np.float32)
    q = np.empty_like(fea)
    for e in range(fea.shape[1]):
        dlo = lo[:, e] - fea[:, e]
        dhi = hi[:, e] - fea[:, e]
        w = Weff[:, e]
        clo = ((v + dlo[:, None] * w) ** 2).sum(1)
        chi = ((v + dhi[:, None] * w) ** 2).sum(1)
        pick_hi = chi < clo
        d = np.where(pick_hi, dhi, dlo)
        q[:, e] = np.where(pick_hi, hi[:, e], lo[:, e])
        v += d[:, None] * w
    return q


def _stage_inputs(fea, W, b):
    import ml_dtypes

    FP8 = ml_dtypes.float8_e4m3fn
    fea = np.asarray(fea, dtype=np.float32)
    W = np.asarray(W, dtype=np.float32)
    b = np.asarray(b, dtype=np.float32)

    # W hi/lo fp8 split (pre-scaled by WS to clear the subnormal range)
    Whi = (W * WS).astype(FP8)
    Wlo = ((W * WS - Whi.astype(np.float32)) * 16.0).astype(FP8)
    Weff = (Whi.astype(np.float32) + Wlo.astype(np.float32) / 16.0) / WS

    q = _diffuse_fp8(fea, Weff).astype(FP8)

    # wt[p, pair, ktile, c]: c 0:4 = Whi, 4:8 = Wlo, 8:16 = 0;
    # e = 256*pair + 128*ktile + p
    wt = np.zeros((128, N_PAIRS, 2, 16), dtype=FP8)
    Whi_r = Whi.reshape(NUM_CLASS, N_PAIRS, 2, 128)  # [c, pair, kt, p]
    Wlo_r = Wlo.reshape(NUM_CLASS, N_PAIRS, 2, 128)
    wt[:, :, :, 0:4] = Whi_r.transpose(3, 1, 2, 0)
    wt[:, :, :, 4:8] = Wlo_r.transpose(3, 1, 2, 0)

    onesv = np.zeros((128, 2, 16), dtype=FP8)
    onesv[:, :, 0] = 1.0
    sones = np.ones((1, 8), dtype=np.float32)
    selw = np.zeros((33, NUM_CLASS), dtype=np.float32)
    selw[0:4, :] = np.eye(NUM_CLASS, dtype=np.float32)
    selw[4:8, :] = np.eye(NUM_CLASS, dtype=np.float32) / 16.0
    selw[32, :] = S * b

    in_maps = []
    for i in range(N_CORES):
        shard = q[i * ROWS : (i + 1) * ROWS, :]
        feaT = np.ascontiguousarray(shard.T).view(np.uint8)
        in_maps.append(
            {
                "feaT": feaT,
                "wt": wt.view(np.uint8),
                "onesv": onesv.view(np.uint8),
                "sones": sones,
                "selw": selw,
            }
        )
    return in_maps


def run(fea, W, b, trace=False):
    from concourse.bass_utils import run_bass_kernel_spmd

    nc = _get_nc()
    in_maps = _stage_inputs(fea, W, b)
    res = run_bass_kernel_spmd(nc, in_maps, core_ids=list(range(N_CORES)), trace=trace)
    out = np.empty((BATCH, NUM_CLASS), dtype=np.float32)
    for i in range(N_CORES):
        out[i * ROWS : (i + 1) * ROWS, :] = res.results[i]["outT"].T
    return out, res


def kernel(fea, W, b):
    out, _ = run(fea, W, b, trace=False)
    return out


# revision 34
# speedup vs baseline: 1.7215x; 1.7215x over previous
"""Trainium2 Bass kernel for BinaryHead: logits = (l2norm(fea) @ W.T + b) * 16.

Sharding: data-parallel over the batch dim across 8 NeuronCores (2048 rows
each).  The host stages each core's shard TRANSPOSED ([emb, batch], a layout
choice) so the embedding/contraction dim lands on SBUF partitions, which is
what the TensorEngine contracts over.  Per core the device kernel streams
e-panel pairs [256e x 2048b]:

  z.T[c, b]   += Wt_chunk.T @ panel            (4-col stationary, panel moving)
  sumsq[b]    += ones.T @ panel**2             (squares on ACT/DVE)

and a small epilogue computes out.T = z.T * (S/sqrt(sumsq)) + S*b on device
(rsqrt via exp(-0.5*ln(ss)+ln(S)) on the scalar engine, class-broadcast via a
k=1 matmul).  The normalization never touches the big tensor.  The first and
last pairs are delivered in four column chunks: the first so the PE starts
early, the last so the epilogue pipelines into the tail of the stream.

Two configs:
  bf16 (default): shard staged as bf16 (halves HBM traffic), z matmuls in
      bf16, sumsq via fp8e4m3 DoubleRow matmuls (one MM contracts both
      panels).  resid-var style error ~2e-6 (scale-relative absmax ~2e-3).
  fp32: full-precision staging streamed as float32r (single-pass PE mode),
      bf16 squares.  scale-relative absmax ~1.1e-4, slower (HBM bound).
"""

import os
from contextlib import ExitStack

import numpy as np

NUM_CLASS = 4
EMB = 2048
BATCH = 16384
N_CORES = 8
ROWS = BATCH // N_CORES  # 2048 rows per core
S = 16.0

N_ETILES = EMB // 128  # 16 e-panels per core
N_BCHUNK = ROWS // 512  # 4 psum-width chunks of the batch

# compute dtype config: "bf16" (fast, default) or "fp32" (f32r matmuls)
DTYPE_CFG = os.environ.get("KERNEL_DTYPE", "bf16")

_CACHE = {}


def _build_nc():
    import concourse.bacc as bacc
    import concourse.mybir as mybir
    import concourse.tile as tile
    from concourse.hw_specs import get_activation_tables

    f32 = mybir.dt.float32
    f32r = mybir.dt.float32r
    bf16 = mybir.dt.bfloat16
    fp8 = mybir.dt.float8e4
    use_bf16 = DTYPE_CFG == "bf16"
    dt_data = bf16 if use_bf16 else f32r
    dt_sq = fp8 if use_bf16 else bf16

    nc = bacc.Bacc(
        "TRN2",
        target_bir_lowering=False,
        debug=False,
        enable_asserts=False,
        num_devices=N_CORES,
    )

    feaT = nc.dram_tensor("feaT", [EMB, ROWS], dt_data, kind="ExternalInput").ap()
    wt = nc.dram_tensor(
        "wt", [128, N_ETILES * NUM_CLASS], dt_data, kind="ExternalInput"
    ).ap()
    if use_bf16:
        onesv = nc.dram_tensor("onesv", [128, 2, 16], fp8, kind="ExternalInput").ap()
    else:
        onesv = nc.dram_tensor("onesv", [128, 1], dt_sq, kind="ExternalInput").ap()
    sones = nc.dram_tensor("sones", [1, NUM_CLASS], f32r, kind="ExternalInput").ap()
    sbias = nc.dram_tensor("sbias", [NUM_CLASS, 1], f32, kind="ExternalInput").ap()
    outT = nc.dram_tensor("outT", [NUM_CLASS, ROWS], f32, kind="ExternalOutput").ap()

    with tile.TileContext(nc) as tc, ExitStack() as ctx:
        pconst = ctx.enter_context(tc.tile_pool(name="pconst", bufs=1))
        pdata = ctx.enter_context(tc.tile_pool(name="pdata", bufs=5))
        psq = ctx.enter_context(tc.tile_pool(name="psq", bufs=4))
        pep = ctx.enter_context(tc.tile_pool(name="pep", bufs=1))
        pz = ctx.enter_context(tc.tile_pool(name="pz", bufs=1, space="PSUM"))
        ps = ctx.enter_context(tc.tile_pool(name="ps", bufs=1, space="PSUM"))

        # wt/ones ride the front of the sync ring (tiny transfers that the
        # first matmuls need); the tail-only consts go through SWDGE
        wt_s = pconst.tile([128, N_ETILES * NUM_CLASS], dt_data)
        nc.sync.dma_start(out=wt_s, in_=wt)
        if use_bf16:
            ones_s = pconst.tile([128, 2, 16], fp8)
        else:
            ones_s = pconst.tile([128, 1], dt_sq)
        nc.sync.dma_start(out=ones_s, in_=onesv)
        sones_s = pconst.tile([1, NUM_CLASS], f32r)
        nc.gpsimd.dma_start(out=sones_s, in_=sones)
        sbias_s = pconst.tile([NUM_CLASS, 1], f32)
        nc.gpsimd.dma_start(out=sbias_s, in_=sbias)
        zero1_s = pconst.tile([1, 1], f32)
        nc.vector.memset(zero1_s, 0.0)
        zero128_s = pconst.tile([128, 1], f32)
        nc.vector.memset(zero128_s, 0.0)
        # rsqrt via exp(-0.5*ln(ss) + ln(S)): folds the *S scale in for free
        lnS_s = pconst.tile([1, 1], f32)
        nc.vector.memset(lnS_s, float(np.log(S)))

        # accumulators: z.T as one 4-bank tensor (PE-only writers), sumsq as
        # four single-bank tensors so the epilogue psum reuse pipelines
        zt_ps = pz.tile([NUM_CLASS, ROWS], f32, tag="zt")
        ss_ps = [
            ps.tile([1, 512], f32, tag="ssrnb", bufs=4, name=f"ss{j}")
            for j in range(N_BCHUNK)
        ]
        rnb = [
            ps.tile([NUM_CLASS, 512], f32, tag="ssrnb", bufs=4, name=f"rnb{j}")
            for j in range(N_BCHUNK)
        ]
        lnss_s = pep.tile([1, ROWS], f32)
        rnorm_s = pep.tile([1, ROWS], f32r)
        z_s = pep.tile([NUM_CLASS, ROWS], f32)
        zr_s = pep.tile([NUM_CLASS, ROWS], f32)
        out_s = pep.tile([NUM_CLASS, ROWS], f32)

        SQ_ACT = {0, 3, 7, 11, 14}  # 5 panels on ACT, 11 on DVE

        def square(a, bsl, xt, x2, ai=None):
            # ACT ~2.0us/panel; DVE bf16 ~1.2us/panel (2x packed mode)
            xin = xt[:, a, bsl] if use_bf16 else xt[:, a, bsl].bitcast(f32)
            if (ai if ai is not None else a) in SQ_ACT:
                nc.scalar.activation(
                    out=x2[:, a, bsl],
                    in_=xin,
                    func=mybir.ActivationFunctionType.Square,
                    bias=zero128_s,
                    scale=1.0,
                )
            else:
                nc.vector.tensor_mul(x2[:, a, bsl], xin, xin)

        def z_mm(t, j, xt, a, start, stop):
            bsl = slice(j * 512, (j + 1) * 512)
            nc.tensor.matmul(
                zt_ps[:, bsl],
                wt_s[:, t * NUM_CLASS : (t + 1) * NUM_CLASS],
                xt[:, a, bsl],
                start=start,
                stop=stop,
            )

        def ss_mm(j, x2, start, stop, p2=0):
            bsl = slice(j * 512, (j + 1) * 512)
            if use_bf16:
                # fp8 DoubleRow: one matmul contracts a panel pair (k=256)
                nc.tensor.matmul(
                    ss_ps[j],
                    ones_s[:, :, 0:1],
                    x2[:, p2 : p2 + 2, bsl],
                    perf_mode=mybir.MatmulPerfMode.DoubleRow,
                    start=start,
                    stop=stop,
                )
            else:
                for a in range(p2, p2 + 2):
                    nc.tensor.matmul(
                        ss_ps[j], ones_s, x2[:, a, bsl], start=start, stop=stop
                    )

        def epilogue_chunk(j):
            # out.T[c,b] = z.T[c,b] * S/sqrt(sumsq[b]) + S*bias[c]
            bsl = slice(j * 512, (j + 1) * 512)
            nc.vector.tensor_copy(z_s[:, bsl], zt_ps[:, bsl])
            nc.scalar.activation(
                out=lnss_s[:, bsl],
                in_=ss_ps[j],
                func=mybir.ActivationFunctionType.Ln,
                bias=zero1_s,
                scale=1.0,
            )
            nc.scalar.activation(
                out=rnorm_s[:, bsl],
                in_=lnss_s[:, bsl],
                func=mybir.ActivationFunctionType.Exp,
                bias=lnS_s,
                scale=-0.5,
            )
            # broadcast S/norm across the 4 class partitions via a k=1 f32r
            # matmul (single-pass PE; reuses a freed sumsq psum bank)
            nc.tensor.matmul(rnb[j], sones_s, rnorm_s[:, bsl], start=True, stop=True)
            nc.vector.tensor_mul(zr_s[:, bsl], z_s[:, bsl], rnb[j])
            nc.vector.tensor_scalar_add(
                out_s[:, bsl], in0=zr_s[:, bsl], scalar1=sbias_s
            )
            nc.sync.dma_start(out=outT[:, bsl], in_=out_s[:, bsl])

        # pre-warm the PE while the first data transfer is in flight:
        # dummy matmuls off memset tiles (no DMA dependency, so they start
        # immediately after the NEFF preamble) into zt_ps (the first real z
        # matmul's start=True resets the bank).  Keeps the HAM clock-gate
        # from re-throttling before real matmuls begin.
        wu_w = pconst.tile([128, NUM_CLASS], dt_data)
        nc.vector.memset(wu_w, 0.0)
        wu_x = pconst.tile([128, 512], dt_data)
        nc.vector.memset(wu_x, 0.0)
        for _ in range(10):
            nc.tensor.matmul(
                zt_ps[:, 0:512],
                wu_w,
                wu_x,
                start=True,
                stop=True,
            )

        # first/last pairs column-chunked; the middle runs as 4-panel quads
        # (2MB transfers amortize per-DMA latency better)
        groups = [(0, 1)] + [
            tuple(range(t, t + 4)) for t in range(2, N_ETILES - 2, 4)
        ] + [(N_ETILES - 2, N_ETILES - 1)]
        for gi, g in enumerate(groups):
            first = gi == 0
            last = gi == len(groups) - 1
            xt = pdata.tile([128, len(g), ROWS], dt_data, tag="xt")
            x2 = psq.tile([128, len(g), ROWS], dt_sq, tag="x2")
            src = feaT[g[0] * 128 : (g[-1] + 1) * 128, :].rearrange(
                "(a p) b -> p a b", p=128
            )
            # alternate the two HWDGE rings (SP and ACT) so transfers overlap
            dma_eng = nc.sync if gi % 2 == 0 else nc.scalar
            if first or last:
                # column-chunked delivery: first pair lets the PE start after
                # a quarter transfer; last pair lets the epilogue overlap the
                # stream tail
                for j in range(N_BCHUNK):
                    bsl = slice(j * 512, (j + 1) * 512)
                    dma_eng.dma_start(out=xt[:, :, bsl], in_=src[:, :, bsl])
            else:
                dma_eng.dma_start(out=xt, in_=src)
            if gi == 1:
                # preload the one ACT table set covering Square+Ln+Exp so no
                # table switch ever lands on the critical path
                nlx_id = list(get_activation_tables(nc.m.arch)).index(
                    "natural_log_exp_and_others"
                )
                nc.scalar.add_instruction(
                    mybir.InstLoadActFuncSet(
                        name=f"I-{nc.next_id()}", act_func_set_id=nlx_id
                    )
                )

            if first or last:
                # per-chunk squares + matmuls so chunk j's chain completes
                # without waiting for the whole pair
                for j in range(N_BCHUNK):
                    bsl = slice(j * 512, (j + 1) * 512)
                    square(0, bsl, xt, x2, ai=g[0])
                    square(1, bsl, xt, x2, ai=g[1])
                    if last:
                        ss_mm(j, x2, start=first, stop=last)
                        z_mm(g[0], j, xt, 0, start=first, stop=False)
                        z_mm(g[1], j, xt, 1, start=False, stop=last)
                        epilogue_chunk(j)
                    else:
                        z_mm(g[0], j, xt, 0, start=first, stop=False)
                        z_mm(g[1], j, xt, 1, start=False, stop=last)
                        ss_mm(j, x2, start=first, stop=last)
            else:
                full = slice(None)
                for a in range(len(g)):
                    square(a, full, xt, x2, ai=g[a])
                # z matmuls first (need only xt), ss after (needs squares)
                for a in range(len(g)):
                    for j in range(N_BCHUNK):
                        z_mm(g[a], j, xt, a, start=False, stop=False)
                for p2 in range(0, len(g), 2):
                    for j in range(N_BCHUNK):
                        ss_mm(j, x2, start=False, stop=False, p2=p2)

    nc.compile()
    return nc


def _get_nc():
    if "nc" not in _CACHE:
        _CACHE["nc"] = _build_nc()
    return _CACHE["nc"]


def _stage_inputs(fea, W, b):
    import ml_dtypes

    np_data = ml_dtypes.bfloat16 if DTYPE_CFG == "bf16" else np.float32
    fea = np.asarray(fea, dtype=np.float32)
    W = np.asarray(W, dtype=np.float32)
    b = np.asarray(b, dtype=np.float32)

    # wt[p, 4t+c] = W[c, 128t+p]
    wt = np.ascontiguousarray(
        W.reshape(NUM_CLASS, N_ETILES, 128).transpose(2, 1, 0).reshape(128, -1)
    ).astype(np_data)
    if DTYPE_CFG == "bf16":
        onesv = np.zeros((128, 2, 16), dtype=ml_dtypes.float8_e4m3)
        onesv[:, :, 0] = 1.0
    else:
        onesv = np.ones((128, 1), dtype=ml_dtypes.bfloat16)
    # the *S scale is folded into the exp(-0.5*ln(ss)+ln(S)) rsqrt, so the
    # class-broadcast matmul uses plain ones
    sones = np.ones((1, NUM_CLASS), dtype=np.float32)
    sbias = (S * b).reshape(NUM_CLASS, 1).astype(np.float32)

    in_maps = []
    for i in range(N_CORES):
        shard = fea[i * ROWS : (i + 1) * ROWS, :]
        feaT = np.ascontiguousarray(shard.T).astype(np_data)
        in_maps.append(
            {"feaT": feaT, "wt": wt, "onesv": onesv, "sones": sones, "sbias": sbias}
        )
    return in_maps


def run(fea, W, b, trace=False):
    from concourse.bass_utils import run_bass_kernel_spmd

    nc = _get_nc()
    in_maps = _stage_inputs(fea, W, b)
    res = run_bass_kernel_spmd(nc, in_maps, core_ids=list(range(N_CORES)), trace=trace)
    out = np.empty((BATCH, NUM_CLASS), dtype=np.float32)
    for i in range(N_CORES):
        out[i * ROWS : (i + 1) * ROWS, :] = res.results[i]["outT"].T
    return out, res


def kernel(fea, W, b):
    out, _ = run(fea, W, b, trace=False)
    return out

